# revision 1
# baseline (speedup 1.0000x reference)
"""GraphSAGE GNN (N=100k, E=600k, D=128, L=4) on 8 Trainium2 NeuronCores.

Strategy (memory-regime):
- Layer 3 of the reference is dead code (its output never reaches node_rep):
  only 3 SAGE layers are computed.
- Nodes sharded contiguously: core c owns dsts [c*12500,(c+1)*12500), padded
  to 12544 rows. Edges sorted by dst on host (graph partitioning), grouped
  per 64-dst group, padded to a uniform K_G tiles of 128 edges -> identical
  SPMD instruction stream on all cores; only data differs.
- Mean-aggregation = PE matmuls: psum[feat,dst] += G_tile.T @ IND_tile where
  G = gathered source rows (indirect DMA, 256B/row bf16) and IND holds
  deg_inv[dst] one-hot columns (resident in SBUF, built on host).
- Layer 0's gather is precomputed on host (x[src]) and streamed sequentially.
- BN (eval) folded into weights/bias on host. Activations kept transposed
  [feat, node] so dense matmuls need no transposes; per-layer JK pooling via
  the ACT accum_out side-output. h tables replicated via AllGather (bf16).
- Final global pool partials [128] per core summed on host + tiny MLP head
  on host (0.1% of FLOPs).
"""
import numpy as np
import ml_dtypes
from contextlib import ExitStack

import concourse.bass as bass
import concourse.tile as tile
import concourse.tile as tile_mod
from concourse import mybir
from concourse.vector_clock import ScopedClock

# ---------------------------------------------------------------------------
# Walrus workaround: this compiler rejects >1 sem wait on CTRL_NO instructions
# (Drain/NoOp). Tile's final drain carries one wait per active proc — split
# them one-per-nop and emit a waitless drain.
def _drain_and_barrier(self, tick_clock, wait_clock):
    probe = self.nc.sync.nop(hint="pre_drain_waits", nofuse=True)
    wait_clock.add_sem_waits(probe.ins, ScopedClock({None: tick_clock.global_clock}))
    si = probe.ins.sync_info
    waits = list(si.on_wait) if si is not None else []
    if len(waits) > 1:
        probe.ins.sync_info = mybir.SyncInfo(on_wait=waits[:1], on_update=[])
        for w in waits[1:]:
            extra = self.nc.sync.nop(hint="pre_drain_waits_x", nofuse=True)
            extra.ins.sync_info = mybir.SyncInfo(on_wait=[w], on_update=[])
    self.nc.sync.drain()
    self.nc.all_engine_barrier()
    assert self.sems is not None
    popped = self.nc._tile_sem_poison_stack.pop()
    assert popped is self._sem_poison
    self.nc.clear_and_free_semaphores(list(self.sems.allocated().values()))
    self.nc.all_engine_barrier()


tile_mod.TileContext._drain_and_barrier = _drain_and_barrier


def _split_multi_waits(nc):
    """This walrus build allows at most ONE sem wait per instruction. Tile
    emits several on some. Split: carrier nops (same engine, program order
    preserved) take all but the last wait."""
    targets = []
    for f in nc.m.functions:
        for blk in f.blocks:
            for inst in blk.instructions:
                si = inst.sync_info
                if si is not None and len(si.on_wait) > 1:
                    targets.append((blk, inst))
    if not targets:
        return
    carriers = {}  # inst name -> list of carrier insts
    created = []
    for blk, inst in targets:
        waits = list(inst.sync_info.on_wait)
        cs = []
        for w in waits[:-1]:
            c = nc.engines[inst.engine].nop(hint="wsplit", nofuse=True)
            c.ins.sync_info = mybir.SyncInfo(on_wait=[w], on_update=[])
            cs.append(c.ins)
            created.append(c.ins.name)
        inst.sync_info = mybir.SyncInfo(
            on_wait=[waits[-1]], on_update=list(inst.sync_info.on_update))
        carriers[inst.name] = cs
    created = set(created)
    for f in nc.m.functions:
        for blk in f.blocks:
            insts = list(blk.instructions)
            new = []
            changed = False
            for inst in insts:
                if inst.name in created:
                    changed = True
                    continue  # remove from tail position
                if inst.name in carriers:
                    new.extend(carriers[inst.name])
                    changed = True
                new.append(inst)
            if changed:
                blk.instructions = new

# ---------------------------------------------------------------------------
N, E, D, L = 100000, 600000, 128, 4
P = 8
NPC = N // P            # 12500 nodes per core
PADN = 12544            # padded to 98*128
V = PADN * P            # padded global table rows
GW = 64                 # dst-group width
NGRP = PADN // GW       # 196 groups per core
BN_EPS = 1e-5
NCHUNK = 25             # 24x512 + 1x256 node chunks
bf16 = mybir.dt.bfloat16
f32 = mybir.dt.float32
i32 = mybir.dt.int32
nbf = ml_dtypes.bfloat16


def _host_prep(x, edge_index):
    src = np.asarray(edge_index[0], dtype=np.int64)
    dst = np.asarray(edge_index[1], dtype=np.int64)
    deg = np.bincount(dst, minlength=N).astype(np.float64)
    deg_inv = np.where(deg > 0, 1.0 / np.maximum(deg, 1.0), 0.0).astype(np.float32)

    order = np.argsort(dst, kind="stable")
    ds = dst[order]
    ss = src[order]
    core_bounds = np.searchsorted(ds, np.arange(P + 1) * NPC)

    # per (core, group) edge counts -> uniform K_G
    maxcnt = 0
    percore = []
    for c in range(P):
        lo, hi = core_bounds[c], core_bounds[c + 1]
        l = (ds[lo:hi] - c * NPC).astype(np.int64)
        s = ss[lo:hi]
        gb = np.searchsorted(l, np.arange(NGRP + 1) * GW)
        cnt = np.diff(gb)
        maxcnt = max(maxcnt, int(cnt.max()))
        percore.append((l, s, gb))
    K_G = (maxcnt + 127) // 128
    T = NGRP * K_G

    srow = (ss // NPC) * PADN + (ss % NPC)  # padded global table row per edge
    xf = np.asarray(x, dtype=np.float32)

    ins = []
    for c in range(P):
        lo, hi = core_bounds[c], core_bounds[c + 1]
        l, s, gb = percore[c]
        sr = srow[order[lo:hi] - 0]  # same order as l/s
        sr = (ss[lo:hi] // NPC) * PADN + (ss[lo:hi] % NPC)

        idx_t = np.zeros((T, 128), np.int32)          # tile, edge-in-tile
        ind_t = np.zeros((T, 128, GW), np.float32)    # tile, edge, dstcol
        xsrc_t = np.zeros((T * 128,), np.int64) - 1   # original src per slot
        for g in range(NGRP):
            e0, e1 = gb[g], gb[g + 1]
            n = e1 - e0
            if n == 0:
                continue
            t0 = g * K_G
            rows = sr[e0:e1]
            cols = (l[e0:e1] - g * GW).astype(np.int64)
            dv = deg_inv[c * NPC + l[e0:e1]] if True else None
            for kk in range((n + 127) // 128):
                a, b = kk * 128, min((kk + 1) * 128, n)
                pp = np.arange(a, b) - a
                idx_t[t0 + kk, pp] = rows[a:b]
                ind_t[t0 + kk, pp, cols[a:b]] = dv[a:b]
                xsrc_t[(t0 + kk) * 128 + pp] = s[e0 + a:e0 + b]

        idx_in = np.ascontiguousarray(idx_t.T)                    # [128, T]
        ind_in = np.ascontiguousarray(
            ind_t.transpose(1, 0, 2).reshape(128, T * GW)).astype(nbf)
        xg = np.zeros((T * 128, D), nbf)
        valid = xsrc_t >= 0
        xg[valid] = xf[xsrc_t[valid]].astype(nbf)

        xT = np.zeros((128, PADN), np.float32)
        xT[:, :NPC] = xf[c * NPC:(c + 1) * NPC].T
        ins.append({
            "idx": idx_in,
            "ind": ind_in,
            "xg": xg,
            "xT": xT.astype(nbf),
        })
    return ins, K_G, deg_inv


def _fold_weights(lin_l_w, lin_l_b, lin_r_w, bn_w, bn_b):
    inv_std = 1.0 / np.sqrt(1.0 + BN_EPS)
    wl, wr, bb = [], [], []
    for l in range(3):
        scale = (np.asarray(bn_w[l], np.float32) * inv_std)
        wl.append((np.asarray(lin_l_w[l], np.float32) * scale[:, None]).T)
        wr.append((np.asarray(lin_r_w[l], np.float32) * scale[:, None]).T)
        bb.append(np.asarray(lin_l_b[l], np.float32) * scale
                  + np.asarray(bn_b[l], np.float32))
    wl = np.stack(wl).astype(nbf)   # [3,128fin,128fout]
    wr = np.stack(wr).astype(nbf)
    bb = np.stack(bb, axis=1).astype(np.float32)  # [128,3]
    return wl, wr, bb


def _build(K_G):
    T = NGRP * K_G
    nc = bass.Bass()
    idx = nc.declare_dram_parameter("idx", [128, T], i32, isOutput=False)
    ind = nc.declare_dram_parameter("ind", [128, T * GW], bf16, isOutput=False)
    xg = nc.declare_dram_parameter("xg", [T * 128, D], bf16, isOutput=False)
    xT = nc.declare_dram_parameter("xT", [128, PADN], bf16, isOutput=False)
    wl = nc.declare_dram_parameter("wl", [3 * 128, 128], bf16, isOutput=False)
    wr = nc.declare_dram_parameter("wr", [3 * 128, 128], bf16, isOutput=False)
    bb = nc.declare_dram_parameter("bb", [128, 3], f32, isOutput=False)
    out = nc.declare_dram_parameter("out", [128, 1], f32, isOutput=True)

    shard = [nc.dram_tensor(f"shard{l}", [PADN, D], bf16) for l in range(2)]
    tables = [nc.dram_tensor(f"table{l}", [V, D], bf16, addr_space="Shared")
              for l in range(2)]

    Relu = mybir.ActivationFunctionType.Relu

    with tile.TileContext(nc) as tc, ExitStack() as ctx:
        res = ctx.enter_context(tc.tile_pool(name="res", bufs=1))
        gp = ctx.enter_context(tc.tile_pool(name="g", bufs=24))
        xgp = ctx.enter_context(tc.tile_pool(name="xgb", bufs=6))
        aggp = ctx.enter_context(tc.tile_pool(name="agg", bufs=3))
        rowp = ctx.enter_context(tc.tile_pool(name="row", bufs=3))
        psg = ctx.enter_context(tc.tile_pool(name="psg", bufs=3, space="PSUM"))
        psh = ctx.enter_context(tc.tile_pool(name="psh", bufs=2, space="PSUM"))
        pst = ctx.enter_context(tc.tile_pool(name="pst", bufs=2, space="PSUM"))

        # ---- residents
        ind_sb = res.tile([128, T * GW], bf16)
        nc.sync.dma_start(ind_sb[:], ind[:])
        idx_sb = res.tile([128, T], i32)
        nc.sync.dma_start(idx_sb[:], idx[:])
        HT = res.tile([128, PADN], bf16)
        nc.sync.dma_start(HT[:], xT[:])
        wl_sb = res.tile([128, 3 * 128], bf16)
        nc.sync.dma_start(wl_sb[:].rearrange("p (l f) -> p l f", l=3),
                          wl[:].rearrange("(l p) f -> p l f", p=128))
        wr_sb = res.tile([128, 3 * 128], bf16)
        nc.sync.dma_start(wr_sb[:].rearrange("p (l f) -> p l f", l=3),
                          wr[:].rearrange("(l p) f -> p l f", p=128))
        bb_sb = res.tile([128, 3], f32)
        nc.sync.dma_start(bb_sb[:], bb[:])
        pool_st = res.tile([128, 3 * NCHUNK + 1], f32)
        ident = res.tile([128, 128], bf16)
        from concourse.masks import make_identity
        make_identity(nc, ident[:])

        for l in range(3):
            table_r = tables[l - 1] if l > 0 else None
            for chunk in range(NCHUNK):
                cs = chunk * 512
                w = 512 if chunk < 24 else 256
                ngr = w // GW
                agg = aggp.tile([128, 512], bf16, tag="agg")
                for gi in range(ngr):
                    g = chunk * 8 + gi
                    ps = psg.tile([128, GW], f32, tag="psg")
                    if l == 0:
                        xgb = xgp.tile([128, K_G * 128], bf16, tag="xgb")
                        nc.sync.dma_start(
                            xgb[:].rearrange("p (j f) -> p j f", j=K_G),
                            xg[g * K_G * 128:(g + 1) * K_G * 128, :]
                            .rearrange("(j p) f -> p j f", p=128))
                    for kk in range(K_G):
                        t = g * K_G + kk
                        if l == 0:
                            G = xgb[:, kk * 128:(kk + 1) * 128]
                        else:
                            Gt = gp.tile([128, 128], bf16, tag="g")
                            nc.gpsimd.indirect_dma_start(
                                out=Gt[:], out_offset=None, in_=table_r[:],
                                in_offset=bass.IndirectOffsetOnAxis(
                                    ap=idx_sb[:, t:t + 1], axis=0))
                            G = Gt[:]
                        nc.tensor.matmul(
                            ps[:], lhsT=G, rhs=ind_sb[:, t * GW:(t + 1) * GW],
                            start=(kk == 0), stop=(kk == K_G - 1))
                    nc.vector.tensor_copy(agg[:, gi * GW:(gi + 1) * GW], ps[:])

                ph = psh.tile([128, 512], f32, tag="psh")
                nc.tensor.matmul(ph[:, :w], lhsT=wl_sb[:, l * 128:(l + 1) * 128],
                                 rhs=agg[:, :w], start=True, stop=False)
                nc.tensor.matmul(ph[:, :w], lhsT=wr_sb[:, l * 128:(l + 1) * 128],
                                 rhs=HT[:, cs:cs + w], start=False, stop=True)
                pcol = pool_st[:, l * NCHUNK + chunk:l * NCHUNK + chunk + 1]
                if chunk < 24:
                    nc.scalar.activation(HT[:, cs:cs + w], ph[:, :w], Relu,
                                         bias=bb_sb[:, l:l + 1], accum_out=pcol)
                else:
                    nc.scalar.activation(HT[:, cs:cs + 212], ph[:, :212], Relu,
                                         bias=bb_sb[:, l:l + 1], accum_out=pcol)
                    nc.scalar.activation(HT[:, cs + 212:cs + 256],
                                         ph[:, 212:256], Relu,
                                         bias=bb_sb[:, l:l + 1])
                if l < 2:
                    row = rowp.tile([128, 512], bf16, tag="row")
                    for j in range(w // 128):
                        pt = pst.tile([128, 128], bf16, tag="pst")
                        nc.tensor.transpose(
                            pt[:], HT[:, cs + j * 128:cs + (j + 1) * 128],
                            ident[:])
                        nc.vector.tensor_copy(row[:, j * 128:(j + 1) * 128],
                                              pt[:])
                    nc.sync.dma_start(
                        shard[l][cs:cs + w, :].rearrange("(j p) f -> p j f",
                                                         p=128),
                        row[:, :w].rearrange("p (j f) -> p j f", f=128))
            if l < 2:
                nc.gpsimd.collective_compute(
                    "AllGather", mybir.AluOpType.bypass,
                    ins=[shard[l][:]], outs=[tables[l][:]],
                    replica_groups=[list(range(P))])

        nc.vector.reduce_sum(pool_st[:, 3 * NCHUNK:], pool_st[:, :3 * NCHUNK],
                             axis=mybir.AxisListType.X)
        outp = res.tile([128, 1], f32)
        nc.vector.tensor_copy(outp[:], pool_st[:, 3 * NCHUNK:])
        nc.sync.dma_start(out[:], outp[:])
    _split_multi_waits(nc)
    return nc


# ---------------------------------------------------------------------------
def _make_runner(nc, n_cores=P):
    import jax
    from jax.sharding import Mesh, PartitionSpec
    try:
        from jax.experimental.shard_map import shard_map
    except ImportError:
        from jax.shard_map import shard_map
    from concourse import bass2jax
    from concourse.bass2jax import _bass_exec_p, partition_id_tensor

    bass2jax.install_neuronx_cc_hook()
    partition_name = nc.partition_id_tensor.name if nc.partition_id_tensor else None
    in_names, out_names, out_avals, zero_outs = [], [], [], []
    for alloc in nc.m.functions[0].allocations:
        if not isinstance(alloc, mybir.MemoryLocationSet):
            continue
        name = alloc.memorylocations[0].name
        if alloc.kind == "ExternalInput":
            if name != partition_name:
                in_names.append(name)
        elif alloc.kind == "ExternalOutput":
            out_names.append(name)
            shape = tuple(alloc.tensor_shape)
            dtype = mybir.dt.np(alloc.dtype)
            out_avals.append(jax.core.ShapedArray(shape, dtype))
            zero_outs.append(np.zeros(shape, dtype))
    n_params = len(in_names)
    in_names_all = list(in_names) + list(out_names)
    if partition_name is not None:
        in_names_all.append(partition_name)

    def _body(*args):
        operands = list(args)
        if partition_name is not None:
            operands.append(partition_id_tensor())
        return tuple(_bass_exec_p.bind(
            *operands, out_avals=tuple(out_avals), in_names=tuple(in_names_all),
            out_names=tuple(out_names), lowering_input_output_aliases=(),
            sim_require_finite=True, sim_require_nnan=True, nc=nc))

    devices = jax.devices()[:n_cores]
    mesh = Mesh(np.asarray(devices), ("core",))
    nspec = n_params + len(out_names)
    sharded = jax.jit(
        shard_map(_body, mesh=mesh,
                  in_specs=(PartitionSpec("core"),) * nspec,
                  out_specs=(PartitionSpec("core"),) * len(out_names),
                  check_rep=False),
        keep_unused=True)

    def run(in_maps):
        per_core = [[np.asarray(m[name]) for name in in_names] for m in in_maps]
        concat_in = [np.concatenate([per_core[c][i] for c in range(n_cores)], axis=0)
                     for i in range(n_params)]
        concat_zeros = [np.zeros((n_cores * z.shape[0], *z.shape[1:]), z.dtype)
                        for z in zero_outs]
        args = concat_in + concat_zeros
        out_arrs = sharded(*args)
        jax.block_until_ready(out_arrs)
        return [{name: np.asarray(out_arrs[i]).reshape(n_cores,
                                                       *out_avals[i].shape)[c]
                 for i, name in enumerate(out_names)}
                for c in range(n_cores)], (sharded, args)
    return run


_CACHE = {}


def kernel(x, lin_l_w, lin_l_b, lin_r_w, bn_w, bn_b,
           fc1_w, fc1_b, fc2_w, fc2_b, fc3_w, fc3_b, edge_index):
    x = np.asarray(x, np.float32)
    per_core, K_G, _ = _host_prep(x, edge_index)
    wlw, wrw, bbw = _fold_weights(lin_l_w, lin_l_b, lin_r_w, bn_w, bn_b)
    wl_in = np.ascontiguousarray(wlw.reshape(3 * 128, 128))
    wr_in = np.ascontiguousarray(wrw.reshape(3 * 128, 128))

    if K_G not in _CACHE:
        nc = _build(K_G)
        _CACHE[K_G] = _make_runner(nc)
    run = _CACHE[K_G]

    in_maps = [{**per_core[c], "wl": wl_in, "wr": wr_in, "bb": bbw}
               for c in range(P)]
    res, _ = run(in_maps)

    g = x.sum(axis=0, dtype=np.float64).astype(np.float32)
    for c in range(P):
        g = g + res[c]["out"][:, 0]

    fc1_w = np.asarray(fc1_w, np.float32); fc1_b = np.asarray(fc1_b, np.float32)
    fc2_w = np.asarray(fc2_w, np.float32); fc2_b = np.asarray(fc2_b, np.float32)
    fc3_w = np.asarray(fc3_w, np.float32); fc3_b = np.asarray(fc3_b, np.float32)
    h = np.maximum(g @ fc1_w.T + fc1_b, 0.0)
    h = np.maximum(h @ fc2_w.T + fc2_b, 0.0)
    o = h @ fc3_w.T + fc3_b
    return o[None, :].astype(np.float32)



# revision 12
# speedup vs baseline: 17.0766x; 17.0766x over previous
"""GraphSAGE GNN (N=100k, E=600k, D=128, L=4) on 8 Trainium2 NeuronCores.

Strategy (memory-regime):
- Layer 3 of the reference is dead code (its output never reaches node_rep):
  only 3 SAGE layers are computed.
- Nodes sharded contiguously by dst: core c owns dsts [c*12500,(c+1)*12500),
  padded to 12544 rows. Edges sorted by dst, grouped per 64-dst group.
- Mean-aggregation = PE matmuls: psum[feat,dst] += G_tile.T @ IND_tile where
  G = gathered source rows and IND holds deg_inv[dst] one-hot columns
  (resident in SBUF, built on host).
- Gathers use the batched SWDGE dma_gather ucode instruction (mlp Q7
  library, loaded via a manually-assembled PSEUDO_LIBRARY_RELOAD): one
  instruction gathers up to 8 tiles (1024 rows), amortizing the ~1us
  per-instruction SWDGE overhead ~8x vs. generic indirect DMA.
- dma_gather idx are int16 and the ucode corrupts addresses for nonzero AP
  offsets or elem_step != elem_size, so the h tables are kept as FOUR
  quarter tensors of 25088 rows (offset-0, 256B-stride gathers only).
  Each group's edges are bucketed by source quarter (cells padded to 128).
- Layer 0's gather is precomputed on host (x[src]) and streamed as a
  partition-major buffer (large descriptors, full DMA bandwidth).
- BN (eval) folded into weights/bias on host. Activations kept transposed
  [feat, node]; per-layer JK pooling via the ACT accum_out side-output.
  h quarter-shards replicated via 4 AllGathers per layer (bf16).
- Final global pool partials [128] per core summed on host + tiny MLP head
  on host (0.1% of FLOPs).
"""
import numpy as np
import ml_dtypes
from contextlib import ExitStack

import concourse.bass as bass
import concourse.bass_isa as bass_isa
import concourse.tile as tile
import concourse.tile as tile_mod
from concourse import mybir, library_config
from concourse.vector_clock import ScopedClock

# ---------------------------------------------------------------------------
# Walrus workaround: this compiler rejects >1 sem wait on CTRL_NO instructions
# (Drain/NoOp). Tile's final drain carries one wait per active proc — split
# them one-per-nop and emit a waitless drain.
def _drain_and_barrier(self, tick_clock, wait_clock):
    probe = self.nc.sync.nop(hint="pre_drain_waits", nofuse=True)
    wait_clock.add_sem_waits(probe.ins, ScopedClock({None: tick_clock.global_clock}))
    si = probe.ins.sync_info
    waits = list(si.on_wait) if si is not None else []
    if len(waits) > 1:
        probe.ins.sync_info = mybir.SyncInfo(on_wait=waits[:1], on_update=[])
        for w in waits[1:]:
            extra = self.nc.sync.nop(hint="pre_drain_waits_x", nofuse=True)
            extra.ins.sync_info = mybir.SyncInfo(on_wait=[w], on_update=[])
    self.nc.sync.drain()
    self.nc.all_engine_barrier()
    assert self.sems is not None
    popped = self.nc._tile_sem_poison_stack.pop()
    assert popped is self._sem_poison
    self.nc.clear_and_free_semaphores(list(self.sems.allocated().values()))
    self.nc.all_engine_barrier()


tile_mod.TileContext._drain_and_barrier = _drain_and_barrier


def _split_multi_waits(nc):
    """This walrus build allows at most ONE sem wait per instruction. Tile
    emits several on some. Split: carrier nops (same engine, program order
    preserved) take all but the last wait."""
    targets = []
    for f in nc.m.functions:
        for blk in f.blocks:
            for inst in blk.instructions:
                si = inst.sync_info
                if si is not None and len(si.on_wait) > 1:
                    targets.append((blk, inst))
    if not targets:
        return
    carriers = {}  # inst name -> list of carrier insts
    created = []
    for blk, inst in targets:
        waits = list(inst.sync_info.on_wait)
        cs = []
        for w in waits[:-1]:
            c = nc.engines[inst.engine].nop(hint="wsplit", nofuse=True)
            c.ins.sync_info = mybir.SyncInfo(on_wait=[w], on_update=[])
            cs.append(c.ins)
            created.append(c.ins.name)
        inst.sync_info = mybir.SyncInfo(
            on_wait=[waits[-1]], on_update=list(inst.sync_info.on_update))
        carriers[inst.name] = cs
    created = set(created)
    for f in nc.m.functions:
        for blk in f.blocks:
            insts = list(blk.instructions)
            new = []
            changed = False
            for inst in insts:
                if inst.name in created:
                    changed = True
                    continue  # remove from tail position
                if inst.name in carriers:
                    new.extend(carriers[inst.name])
                    changed = True
                new.append(inst)
            if changed:
                blk.instructions = new


def _fill_reload_bytes(nc):
    """Assemble PSEUDO_LIBRARY_RELOAD_INDEX bytes (walrus rejects the
    empty-instr pseudo instruction with 'ISA wrong length')."""
    isa = nc.isa
    for f in nc.m.functions:
        for blk in f.blocks:
            for inst in blk.instructions:
                if isinstance(inst, bass_isa.InstPseudoReloadLibraryIndex):
                    words, _ = bass_isa.isa_struct(
                        isa, isa.Opcode.NEURON_ISA_TPB_OPCODE_PSEUDO_INST,
                        {"pseudo_opcode": 2, "lib_index": inst.lib_index},
                        struct_name=(
                            "NEURON_ISA_TPB_PSEUDO_LIBRARY_RELOAD_INDEX_STRUCT"))
                    inst.instr = words


# ---------------------------------------------------------------------------
N, E, D = 100000, 600000, 128
P = 8
NPC = N // P            # 12500 nodes per core
PADN = 12544            # padded to 98*128
GW = 64                 # dst-group width
NGRP = PADN // GW       # 196 groups per core
NCHUNK = 25             # 24x512 + 1x256 node chunks
GPC = 8                 # groups per chunk
NQ = 4                  # source quarters (int16 idx reach)
QSIZES = (3200, 3200, 3072, 3072)   # 128-aligned, sum = PADN; 8*q <= 32767
QSTART = (0, 3200, 6400, 9472)
MAXTPI = 8              # max tiles per dma_gather (1024 idx ucode limit)
BN_EPS = 1e-5
bf16 = mybir.dt.bfloat16
f32 = mybir.dt.float32
i16 = mybir.dt.int16
nbf = ml_dtypes.bfloat16


def _plan_chunks():
    """Static chunk layout: list of (group_lo, group_hi, col_lo, width)."""
    out = []
    for ci in range(NCHUNK):
        glo = ci * GPC
        ghi = min(glo + GPC, NGRP)
        out.append((glo, ghi, glo * GW, (ghi - glo) * GW))
    return out


def _host_prep(x, edge_index):
    src = np.asarray(edge_index[0], dtype=np.int64)
    dst = np.asarray(edge_index[1], dtype=np.int64)
    deg = np.bincount(dst, minlength=N).astype(np.float64)
    deg_inv = np.where(deg > 0, 1.0 / np.maximum(deg, 1.0), 0.0).astype(np.float32)

    core = dst // NPC
    l = dst - core * NPC
    g = l // GW
    col = l - g * GW
    sc = src // NPC
    sl = src - sc * NPC
    q = np.searchsorted(np.asarray(QSTART), sl, side="right") - 1
    qs = np.asarray(QSIZES)[q]
    rel = sc * qs + (sl - np.asarray(QSTART)[q])  # row within quarter table

    # global per-(core, g, q) counts -> shared tile structure
    cell = (g * NQ + q).astype(np.int64)
    ckey = core * (NGRP * NQ) + cell
    counts = np.bincount(ckey, minlength=P * NGRP * NQ).reshape(P, NGRP, NQ)
    K = np.maximum(0, (counts.max(axis=0) + 127) // 128)   # ceil
    K = ((counts + 127) // 128).max(axis=0)                # [NGRP, NQ]

    chunks = _plan_chunks()
    # tile order: chunk -> quarter -> group
    tile_of_cell = np.full((NGRP, NQ), -1, np.int64)
    chunk_meta = []
    t = 0
    for ci, (glo, ghi, _, _) in enumerate(chunks):
        tbase = t
        pieces = []   # (quarter, tile_off_in_chunk, ntiles)
        mm = []       # (tile_off_in_chunk, colbase, start, stop)
        gtot = {gg: int(K[gg].sum()) for gg in range(glo, ghi)}
        gseen = {gg: 0 for gg in range(glo, ghi)}
        for qq in range(NQ):
            run0 = t
            for gg in range(glo, ghi):
                k = int(K[gg, qq])
                if k == 0:
                    continue
                tile_of_cell[gg, qq] = t
                for kk in range(k):
                    st = gseen[gg] == 0
                    gseen[gg] += 1
                    sp = gseen[gg] == gtot[gg]
                    mm.append((t - tbase, (gg - glo) * GW, st, sp))
                    t += 1
            run = t - run0
            off = 0
            while off < run:
                n = min(MAXTPI, run - off)
                pieces.append((qq, run0 - tbase + off, n))
                off += n
        chunk_meta.append({"tbase": tbase, "ntiles": t - tbase,
                           "pieces": pieces, "mm": mm})
    T = t

    # per-core slot assignment
    xf = np.asarray(x, dtype=np.float32)
    xbf = xf.astype(nbf)
    dv_e = deg_inv[dst].astype(np.float32)

    ins = []
    for c in range(P):
        m = core == c
        eg, ecol, eq, erel, esrc, edv = g[m], col[m], q[m], rel[m], src[m], dv_e[m]
        ecell = eg * NQ + eq
        order = np.argsort(ecell, kind="stable")
        ecell_s = ecell[order]
        # position within cell
        cb = np.searchsorted(ecell_s, np.arange(NGRP * NQ))
        pos = np.arange(len(ecell_s)) - cb[ecell_s]
        slot_base = (tile_of_cell.reshape(-1) * 128)
        slot = slot_base[ecell_s] + pos
        assert (pos < 128 * K.reshape(-1)[ecell_s]).all()

        sp = slot % 128
        st = slot // 128
        idx16 = np.zeros((16, 8 * T), np.int16)
        idx16[slot % 16, slot // 16] = erel[order]
        idx_in = np.ascontiguousarray(np.tile(idx16, (8, 1)))

        ind = np.zeros((128, T * GW), np.float32)
        ind[sp, st * GW + ecol[order]] = edv[order]
        ind_in = ind.astype(nbf)

        xg = np.zeros((128, T, 128), nbf)
        xg[sp, st, :] = xbf[esrc[order]]
        xg_in = np.ascontiguousarray(xg.reshape(128, T * 128))

        xT = np.zeros((128, PADN), np.float32)
        xT[:, :NPC] = xf[c * NPC:(c + 1) * NPC].T
        ins.append({
            "idx": idx_in,
            "ind": ind_in,
            "xg": xg_in,
            "xT": xT.astype(nbf),
        })
    plan = {"T": T, "chunks": chunks, "meta": chunk_meta}
    return ins, plan, deg_inv


def _fold_weights(lin_l_w, lin_l_b, lin_r_w, bn_w, bn_b):
    inv_std = 1.0 / np.sqrt(1.0 + BN_EPS)
    wl, wr, bb = [], [], []
    for li in range(3):
        scale = (np.asarray(bn_w[li], np.float32) * inv_std)
        wl.append((np.asarray(lin_l_w[li], np.float32) * scale[:, None]).T)
        wr.append((np.asarray(lin_r_w[li], np.float32) * scale[:, None]).T)
        bb.append(np.asarray(lin_l_b[li], np.float32) * scale
                  + np.asarray(bn_b[li], np.float32))
    wl = np.stack(wl).astype(nbf)   # [3,128fin,128fout]
    wr = np.stack(wr).astype(nbf)
    bb = np.stack(bb, axis=1).astype(np.float32)  # [128,3]
    return wl, wr, bb


def _quarter_pieces(cs, w):
    """Split chunk rows [cs, cs+w) at quarter boundaries.
    Returns (quarter, row_in_quarter, count, col_off)."""
    out = []
    r = cs
    while r < cs + w:
        qq = max(i for i in range(NQ) if QSTART[i] <= r)
        hi = min(QSTART[qq] + QSIZES[qq], cs + w)
        out.append((qq, r - QSTART[qq], hi - r, r - cs))
        r = hi
    return out


def _build(plan, debug=False):
    T = plan["T"]
    chunks = plan["chunks"]
    meta = plan["meta"]
    nc = bass.Bass()
    idx = nc.declare_dram_parameter("idx", [128, 8 * T], i16, isOutput=False)
    ind = nc.declare_dram_parameter("ind", [128, T * GW], bf16, isOutput=False)
    xg = nc.declare_dram_parameter("xg", [128, T * 128], bf16, isOutput=False)
    xT = nc.declare_dram_parameter("xT", [128, PADN], bf16, isOutput=False)
    wl = nc.declare_dram_parameter("wl", [3 * 128, 128], bf16, isOutput=False)
    wr = nc.declare_dram_parameter("wr", [3 * 128, 128], bf16, isOutput=False)
    bb = nc.declare_dram_parameter("bb", [128, 3], f32, isOutput=False)
    out = nc.declare_dram_parameter("out", [128, 1], f32, isOutput=True)
    htdbg = [nc.declare_dram_parameter(f"ht{li}", [128, PADN], bf16,
                                       isOutput=True)
             for li in range(3)] if debug else None

    shard = [[nc.dram_tensor(f"shard{li}_{qq}", [QSIZES[qq], D], bf16)
              for qq in range(NQ)] for li in range(2)]
    tables = [[nc.dram_tensor(f"table{li}_{qq}", [P * QSIZES[qq], D], bf16,
                              addr_space="Shared")
               for qq in range(NQ)] for li in range(2)]

    Relu = mybir.ActivationFunctionType.Relu
    GMAX = max(m["ntiles"] for m in meta)

    with tile.TileContext(nc) as tc, ExitStack() as ctx:
        nc.gpsimd.load_library(library_config.mlp)
        nreg = {}  # one register per distinct num_idxs value
        for m in meta:
            for (_, _, n) in m["pieces"]:
                if n * 128 not in nreg:
                    nreg[n * 128] = nc.gpsimd.to_reg(n * 128)
        res = ctx.enter_context(tc.tile_pool(name="res", bufs=1))
        gp = ctx.enter_context(tc.tile_pool(name="g", bufs=3))
        aggp = ctx.enter_context(tc.tile_pool(name="agg", bufs=2))
        rowp = ctx.enter_context(tc.tile_pool(name="row", bufs=2))
        psg = ctx.enter_context(tc.tile_pool(name="psg", bufs=2, space="PSUM"))
        psh = ctx.enter_context(tc.tile_pool(name="psh", bufs=2, space="PSUM"))
        pst = ctx.enter_context(tc.tile_pool(name="pst", bufs=2, space="PSUM"))

        # ---- residents
        ind_sb = res.tile([128, T * GW], bf16)
        nc.sync.dma_start(ind_sb[:], ind[:])
        idx_sb = res.tile([128, 8 * T], i16)
        nc.sync.dma_start(idx_sb[:], idx[:])
        HT = res.tile([128, PADN], bf16)
        nc.sync.dma_start(HT[:], xT[:])
        wl_sb = res.tile([128, 3 * 128], bf16)
        nc.sync.dma_start(wl_sb[:].rearrange("p (l f) -> p l f", l=3),
                          wl[:].rearrange("(l p) f -> p l f", p=128))
        wr_sb = res.tile([128, 3 * 128], bf16)
        nc.sync.dma_start(wr_sb[:].rearrange("p (l f) -> p l f", l=3),
                          wr[:].rearrange("(l p) f -> p l f", p=128))
        bb_sb = res.tile([128, 3], f32)
        nc.sync.dma_start(bb_sb[:], bb[:])
        pool_st = res.tile([128, 3 * NCHUNK + 1], f32)
        ident = res.tile([128, 128], bf16)
        from concourse.masks import make_identity
        make_identity(nc, ident[:])

        for li in range(3):
            for ci in range(NCHUNK):
                glo, ghi, cs, w = chunks[ci]
                m = meta[ci]
                tb, nt = m["tbase"], m["ntiles"]
                G = gp.tile([128, GMAX * 128], bf16, tag="g")
                if li == 0:
                    nc.sync.dma_start(G[:, :nt * 128],
                                      xg[:, tb * 128:(tb + nt) * 128])
                else:
                    for (qq, toff, n) in m["pieces"]:
                        nc.gpsimd.dma_gather(
                            out_ap=G[:, toff * 128:(toff + n) * 128]
                            .rearrange("p (j f) -> p j f", f=128),
                            in_ap=tables[li - 1][qq][:],
                            idxs_ap=idx_sb[:, 8 * (tb + toff):8 * (tb + toff + n)],
                            num_idxs=n * 128, num_idxs_reg=nreg[n * 128],
                            elem_size=128)
                ps = psg.tile([128, 512], f32, tag="psg")
                # PSUM accumulation chains must be consecutive per region:
                # issue group-major (stable keeps start->stop order)
                for (toff, cb, st, sp) in sorted(m["mm"], key=lambda z: z[1]):
                    nc.tensor.matmul(
                        ps[:, cb:cb + GW],
                        lhsT=G[:, toff * 128:(toff + 1) * 128],
                        rhs=ind_sb[:, (tb + toff) * GW:(tb + toff + 1) * GW],
                        start=st, stop=sp)
                agg = aggp.tile([128, 512], bf16, tag="agg")
                nc.vector.tensor_copy(agg[:, :w], ps[:, :w])

                ph = psh.tile([128, 512], f32, tag="psh")
                nc.tensor.matmul(ph[:, :w], lhsT=wl_sb[:, li * 128:(li + 1) * 128],
                                 rhs=agg[:, :w], start=True, stop=False)
                nc.tensor.matmul(ph[:, :w], lhsT=wr_sb[:, li * 128:(li + 1) * 128],
                                 rhs=HT[:, cs:cs + w], start=False, stop=True)
                pcol = pool_st[:, li * NCHUNK + ci:li * NCHUNK + ci + 1]
                if ci < NCHUNK - 1:
                    nc.scalar.activation(HT[:, cs:cs + w], ph[:, :w], Relu,
                                         bias=bb_sb[:, li:li + 1], accum_out=pcol)
                else:
                    nv = NPC - cs  # valid columns in last chunk
                    nc.scalar.activation(HT[:, cs:cs + nv], ph[:, :nv], Relu,
                                         bias=bb_sb[:, li:li + 1], accum_out=pcol)
                    nc.scalar.activation(HT[:, cs + nv:cs + w],
                                         ph[:, nv:w], Relu,
                                         bias=bb_sb[:, li:li + 1])
                if li < 2:
                    row = rowp.tile([128, 512], bf16, tag="row")
                    for j in range(w // 128):
                        pt = pst.tile([128, 128], bf16, tag="pst")
                        nc.tensor.transpose(
                            pt[:], HT[:, cs + j * 128:cs + (j + 1) * 128],
                            ident[:])
                        nc.vector.tensor_copy(row[:, j * 128:(j + 1) * 128],
                                              pt[:])
                    for (qq, qr, cnt, coff) in _quarter_pieces(cs, w):
                        nc.sync.dma_start(
                            shard[li][qq][qr:qr + cnt, :]
                            .rearrange("(j p) f -> p j f", p=128),
                            row[:, coff:coff + cnt]
                            .rearrange("p (j f) -> p j f", f=128))
            if debug:
                nc.sync.dma_start(htdbg[li][:], HT[:])
            if li < 2:
                for qq in range(NQ):
                    nc.gpsimd.collective_compute(
                        "AllGather", mybir.AluOpType.bypass,
                        ins=[shard[li][qq][:]], outs=[tables[li][qq][:]],
                        replica_groups=[list(range(P))])

        nc.vector.reduce_sum(pool_st[:, 3 * NCHUNK:], pool_st[:, :3 * NCHUNK],
                             axis=mybir.AxisListType.X)
        outp = res.tile([128, 1], f32)
        nc.vector.tensor_copy(outp[:], pool_st[:, 3 * NCHUNK:])
        nc.sync.dma_start(out[:], outp[:])
    _split_multi_waits(nc)
    _fill_reload_bytes(nc)
    return nc


# ---------------------------------------------------------------------------
def _make_runner(nc, n_cores=P):
    import jax
    from jax.sharding import Mesh, PartitionSpec
    try:
        from jax.experimental.shard_map import shard_map
    except ImportError:
        from jax.shard_map import shard_map
    from concourse import bass2jax
    from concourse.bass2jax import _bass_exec_p, partition_id_tensor

    bass2jax.install_neuronx_cc_hook()
    partition_name = nc.partition_id_tensor.name if nc.partition_id_tensor else None
    in_names, out_names, out_avals, zero_outs = [], [], [], []
    for alloc in nc.m.functions[0].allocations:
        if not isinstance(alloc, mybir.MemoryLocationSet):
            continue
        name = alloc.memorylocations[0].name
        if alloc.kind == "ExternalInput":
            if name != partition_name:
                in_names.append(name)
        elif alloc.kind == "ExternalOutput":
            out_names.append(name)
            shape = tuple(alloc.tensor_shape)
            dtype = mybir.dt.np(alloc.dtype)
            out_avals.append(jax.core.ShapedArray(shape, dtype))
            zero_outs.append(np.zeros(shape, dtype))
    n_params = len(in_names)
    in_names_all = list(in_names) + list(out_names)
    if partition_name is not None:
        in_names_all.append(partition_name)

    def _body(*args):
        operands = list(args)
        if partition_name is not None:
            operands.append(partition_id_tensor())
        return tuple(_bass_exec_p.bind(
            *operands, out_avals=tuple(out_avals), in_names=tuple(in_names_all),
            out_names=tuple(out_names), lowering_input_output_aliases=(),
            sim_require_finite=True, sim_require_nnan=True, nc=nc))

    devices = jax.devices()[:n_cores]
    mesh = Mesh(np.asarray(devices), ("core",))
    nspec = n_params + len(out_names)
    sharded = jax.jit(
        shard_map(_body, mesh=mesh,
                  in_specs=(PartitionSpec("core"),) * nspec,
                  out_specs=(PartitionSpec("core"),) * len(out_names),
                  check_rep=False),
        keep_unused=True)

    def run(in_maps):
        per_core = [[np.asarray(m[name]) for name in in_names] for m in in_maps]
        concat_in = [np.concatenate([per_core[c][i] for c in range(n_cores)], axis=0)
                     for i in range(n_params)]
        concat_zeros = [np.zeros((n_cores * z.shape[0], *z.shape[1:]), z.dtype)
                        for z in zero_outs]
        args = concat_in + concat_zeros
        out_arrs = sharded(*args)
        jax.block_until_ready(out_arrs)
        return [{name: np.asarray(out_arrs[i]).reshape(n_cores,
                                                       *out_avals[i].shape)[c]
                 for i, name in enumerate(out_names)}
                for c in range(n_cores)], (sharded, args)
    return run


_CACHE = {}


def kernel(x, lin_l_w, lin_l_b, lin_r_w, bn_w, bn_b,
           fc1_w, fc1_b, fc2_w, fc2_b, fc3_w, fc3_b, edge_index):
    x = np.asarray(x, np.float32)
    per_core, plan, _ = _host_prep(x, edge_index)
    wlw, wrw, bbw = _fold_weights(lin_l_w, lin_l_b, lin_r_w, bn_w, bn_b)
    wl_in = np.ascontiguousarray(wlw.reshape(3 * 128, 128))
    wr_in = np.ascontiguousarray(wrw.reshape(3 * 128, 128))

    key = (plan["T"],) + tuple(
        (m["tbase"],) + tuple(m["pieces"]) for m in plan["meta"])
    if key not in _CACHE:
        nc = _build(plan)
        _CACHE[key] = _make_runner(nc)
    run = _CACHE[key]

    in_maps = [{**per_core[c], "wl": wl_in, "wr": wr_in, "bb": bbw}
               for c in range(P)]
    res, _ = run(in_maps)

    g = x.sum(axis=0, dtype=np.float64).astype(np.float32)
    for c in range(P):
        g = g + res[c]["out"][:, 0]

    fc1_w = np.asarray(fc1_w, np.float32); fc1_b = np.asarray(fc1_b, np.float32)
    fc2_w = np.asarray(fc2_w, np.float32); fc2_b = np.asarray(fc2_b, np.float32)
    fc3_w = np.asarray(fc3_w, np.float32); fc3_b = np.asarray(fc3_b, np.float32)
    h = np.maximum(g @ fc1_w.T + fc1_b, 0.0)
    h = np.maximum(h @ fc2_w.T + fc2_b, 0.0)
    o = h @ fc3_w.T + fc3_b
    return o[None, :].astype(np.float32)


# revision 15
# speedup vs baseline: 18.0703x; 1.0582x over previous
"""GraphSAGE GNN (N=100k, E=600k, D=128, L=4) on 8 Trainium2 NeuronCores.

Strategy (memory-regime):
- Layer 3 of the reference is dead code (its output never reaches node_rep):
  only 3 SAGE layers are computed.
- Nodes sharded contiguously by dst: core c owns dsts [c*12500,(c+1)*12500),
  padded to 12544 rows. Edges sorted by dst, grouped per 64-dst group.
- Mean-aggregation = PE matmuls: psum[feat,dst] += G_tile.T @ IND_tile where
  G = gathered source rows and IND holds deg_inv[dst] one-hot columns
  (resident in SBUF, built on host).
- Gathers use the batched SWDGE dma_gather ucode instruction (mlp Q7
  library, loaded via a manually-assembled PSEUDO_LIBRARY_RELOAD): one
  instruction gathers up to 8 tiles (1024 rows), amortizing the ~1us
  per-instruction SWDGE overhead ~8x vs. generic indirect DMA.
- dma_gather idx are int16 and the ucode corrupts addresses for nonzero AP
  offsets or elem_step != elem_size, so the h tables are kept as FOUR
  quarter tensors of 25088 rows (offset-0, 256B-stride gathers only).
  Each group's edges are bucketed by source quarter (cells padded to 128).
- Layer 0's gather is precomputed on host (x[src]) and streamed as a
  partition-major buffer (large descriptors, full DMA bandwidth).
- BN (eval) folded into weights/bias on host. Activations kept transposed
  [feat, node]; per-layer JK pooling via the ACT accum_out side-output.
  h quarter-shards replicated via 4 AllGathers per layer (bf16).
- Final global pool partials [128] per core summed on host + tiny MLP head
  on host (0.1% of FLOPs).
"""
import numpy as np
import ml_dtypes
from contextlib import ExitStack

import concourse.bass as bass
import concourse.bass_isa as bass_isa
import concourse.tile as tile
import concourse.tile as tile_mod
from concourse import mybir, library_config
from concourse.vector_clock import ScopedClock

# ---------------------------------------------------------------------------
# Walrus workaround: this compiler rejects >1 sem wait on CTRL_NO instructions
# (Drain/NoOp). Tile's final drain carries one wait per active proc — split
# them one-per-nop and emit a waitless drain.
def _drain_and_barrier(self, tick_clock, wait_clock):
    probe = self.nc.sync.nop(hint="pre_drain_waits", nofuse=True)
    wait_clock.add_sem_waits(probe.ins, ScopedClock({None: tick_clock.global_clock}))
    si = probe.ins.sync_info
    waits = list(si.on_wait) if si is not None else []
    if len(waits) > 1:
        probe.ins.sync_info = mybir.SyncInfo(on_wait=waits[:1], on_update=[])
        for w in waits[1:]:
            extra = self.nc.sync.nop(hint="pre_drain_waits_x", nofuse=True)
            extra.ins.sync_info = mybir.SyncInfo(on_wait=[w], on_update=[])
    self.nc.sync.drain()
    self.nc.all_engine_barrier()
    assert self.sems is not None
    popped = self.nc._tile_sem_poison_stack.pop()
    assert popped is self._sem_poison
    self.nc.clear_and_free_semaphores(list(self.sems.allocated().values()))
    self.nc.all_engine_barrier()


tile_mod.TileContext._drain_and_barrier = _drain_and_barrier


def _split_multi_waits(nc):
    """This walrus build allows at most ONE sem wait per instruction. Tile
    emits several on some. Split: carrier nops (same engine, program order
    preserved) take all but the last wait."""
    targets = []
    for f in nc.m.functions:
        for blk in f.blocks:
            for inst in blk.instructions:
                si = inst.sync_info
                if si is not None and len(si.on_wait) > 1:
                    targets.append((blk, inst))
    if not targets:
        return
    carriers = {}  # inst name -> list of carrier insts
    created = []
    for blk, inst in targets:
        waits = list(inst.sync_info.on_wait)
        cs = []
        for w in waits[:-1]:
            c = nc.engines[inst.engine].nop(hint="wsplit", nofuse=True)
            c.ins.sync_info = mybir.SyncInfo(on_wait=[w], on_update=[])
            cs.append(c.ins)
            created.append(c.ins.name)
        inst.sync_info = mybir.SyncInfo(
            on_wait=[waits[-1]], on_update=list(inst.sync_info.on_update))
        carriers[inst.name] = cs
    created = set(created)
    for f in nc.m.functions:
        for blk in f.blocks:
            insts = list(blk.instructions)
            new = []
            changed = False
            for inst in insts:
                if inst.name in created:
                    changed = True
                    continue  # remove from tail position
                if inst.name in carriers:
                    new.extend(carriers[inst.name])
                    changed = True
                new.append(inst)
            if changed:
                blk.instructions = new


def _fill_reload_bytes(nc):
    """Assemble PSEUDO_LIBRARY_RELOAD_INDEX bytes (walrus rejects the
    empty-instr pseudo instruction with 'ISA wrong length')."""
    isa = nc.isa
    for f in nc.m.functions:
        for blk in f.blocks:
            for inst in blk.instructions:
                if isinstance(inst, bass_isa.InstPseudoReloadLibraryIndex):
                    words, _ = bass_isa.isa_struct(
                        isa, isa.Opcode.NEURON_ISA_TPB_OPCODE_PSEUDO_INST,
                        {"pseudo_opcode": 2, "lib_index": inst.lib_index},
                        struct_name=(
                            "NEURON_ISA_TPB_PSEUDO_LIBRARY_RELOAD_INDEX_STRUCT"))
                    inst.instr = words


# ---------------------------------------------------------------------------
N, E, D = 100000, 600000, 128
P = 8
NPC = N // P            # 12500 nodes per core
PADN = 12544            # padded to 98*128
GW = 64                 # dst-group width
NGRP = PADN // GW       # 196 groups per core
NCHUNK = 25             # 24x512 + 1x256 node chunks
GPC = 8                 # groups per chunk
NQ = 4                  # source quarters (int16 idx reach)
QSIZES = (3200, 3200, 3072, 3072)   # 128-aligned, sum = PADN; 8*q <= 32767
QSTART = (0, 3200, 6400, 9472)
MAXTPI = 8              # max tiles per dma_gather (1024 idx ucode limit)
BN_EPS = 1e-5
bf16 = mybir.dt.bfloat16
f32 = mybir.dt.float32
i16 = mybir.dt.int16
nbf = ml_dtypes.bfloat16


def _plan_chunks():
    """Static chunk layout: list of (group_lo, group_hi, col_lo, width)."""
    out = []
    for ci in range(NCHUNK):
        glo = ci * GPC
        ghi = min(glo + GPC, NGRP)
        out.append((glo, ghi, glo * GW, (ghi - glo) * GW))
    return out


def _host_prep(x, edge_index):
    src = np.asarray(edge_index[0], dtype=np.int64)
    dst = np.asarray(edge_index[1], dtype=np.int64)
    deg = np.bincount(dst, minlength=N).astype(np.float64)
    deg_inv = np.where(deg > 0, 1.0 / np.maximum(deg, 1.0), 0.0).astype(np.float32)

    core = dst // NPC
    l = dst - core * NPC
    g = l // GW
    col = l - g * GW
    sc = src // NPC
    sl = src - sc * NPC
    q = np.searchsorted(np.asarray(QSTART), sl, side="right") - 1
    qs = np.asarray(QSIZES)[q]
    rel = sc * qs + (sl - np.asarray(QSTART)[q])  # row within quarter table

    # global per-(core, g, q) counts -> shared tile structure
    cell = (g * NQ + q).astype(np.int64)
    ckey = core * (NGRP * NQ) + cell
    counts = np.bincount(ckey, minlength=P * NGRP * NQ).reshape(P, NGRP, NQ)
    K = np.maximum(0, (counts.max(axis=0) + 127) // 128)   # ceil
    K = ((counts + 127) // 128).max(axis=0)                # [NGRP, NQ]

    chunks = _plan_chunks()
    # tile order: chunk -> quarter -> group
    tile_of_cell = np.full((NGRP, NQ), -1, np.int64)
    chunk_meta = []
    t = 0
    for ci, (glo, ghi, _, _) in enumerate(chunks):
        tbase = t
        pieces = []   # (quarter, tile_off_in_chunk, ntiles)
        mm = []       # (tile_off_in_chunk, colbase, start, stop)
        gtot = {gg: int(K[gg].sum()) for gg in range(glo, ghi)}
        gseen = {gg: 0 for gg in range(glo, ghi)}
        for qq in range(NQ):
            run0 = t
            for gg in range(glo, ghi):
                k = int(K[gg, qq])
                if k == 0:
                    continue
                tile_of_cell[gg, qq] = t
                for kk in range(k):
                    st = gseen[gg] == 0
                    gseen[gg] += 1
                    sp = gseen[gg] == gtot[gg]
                    mm.append((t - tbase, (gg - glo) * GW, st, sp))
                    t += 1
            run = t - run0
            off = 0
            while off < run:
                n = min(MAXTPI, run - off)
                pieces.append((qq, run0 - tbase + off, n))
                off += n
        chunk_meta.append({"tbase": tbase, "ntiles": t - tbase,
                           "pieces": pieces, "mm": mm})
    T = t

    # per-core slot assignment
    xf = np.asarray(x, dtype=np.float32)
    xbf = xf.astype(nbf)
    dv_e = deg_inv[dst].astype(np.float32)

    ins = []
    for c in range(P):
        m = core == c
        eg, ecol, eq, erel, esrc, edv = g[m], col[m], q[m], rel[m], src[m], dv_e[m]
        ecell = eg * NQ + eq
        order = np.argsort(ecell, kind="stable")
        ecell_s = ecell[order]
        # position within cell
        cb = np.searchsorted(ecell_s, np.arange(NGRP * NQ))
        pos = np.arange(len(ecell_s)) - cb[ecell_s]
        slot_base = (tile_of_cell.reshape(-1) * 128)
        slot = slot_base[ecell_s] + pos
        assert (pos < 128 * K.reshape(-1)[ecell_s]).all()

        sp = slot % 128
        st = slot // 128
        idx16 = np.zeros((16, 8 * T), np.int16)
        idx16[slot % 16, slot // 16] = erel[order]
        idx_in = np.ascontiguousarray(np.tile(idx16, (8, 1)))

        ind = np.zeros((128, T * GW), np.float32)
        ind[sp, st * GW + ecol[order]] = edv[order]
        ind_in = ind.astype(nbf)

        xg = np.zeros((128, T, 128), nbf)
        xg[sp, st, :] = xbf[esrc[order]]
        xg_in = np.ascontiguousarray(xg.reshape(128, T * 128))

        xT = np.zeros((128, PADN), np.float32)
        xT[:, :NPC] = xf[c * NPC:(c + 1) * NPC].T
        ins.append({
            "idx": idx_in,
            "ind": ind_in,
            "xg": xg_in,
            "xT": xT.astype(nbf),
        })
    plan = {"T": T, "chunks": chunks, "meta": chunk_meta}
    return ins, plan, deg_inv


def _fold_weights(lin_l_w, lin_l_b, lin_r_w, bn_w, bn_b):
    inv_std = 1.0 / np.sqrt(1.0 + BN_EPS)
    wl, wr, bb = [], [], []
    for li in range(3):
        scale = (np.asarray(bn_w[li], np.float32) * inv_std)
        wl.append((np.asarray(lin_l_w[li], np.float32) * scale[:, None]).T)
        wr.append((np.asarray(lin_r_w[li], np.float32) * scale[:, None]).T)
        bb.append(np.asarray(lin_l_b[li], np.float32) * scale
                  + np.asarray(bn_b[li], np.float32))
    wl = np.stack(wl).astype(nbf)   # [3,128fin,128fout]
    wr = np.stack(wr).astype(nbf)
    bb = np.stack(bb, axis=1).astype(np.float32)  # [128,3]
    return wl, wr, bb


def _quarter_pieces(cs, w):
    """Split chunk rows [cs, cs+w) at quarter boundaries.
    Returns (quarter, row_in_quarter, count, col_off)."""
    out = []
    r = cs
    while r < cs + w:
        qq = max(i for i in range(NQ) if QSTART[i] <= r)
        hi = min(QSTART[qq] + QSIZES[qq], cs + w)
        out.append((qq, r - QSTART[qq], hi - r, r - cs))
        r = hi
    return out


def _build(plan, debug=False, sim_local_collectives=False):
    T = plan["T"]
    chunks = plan["chunks"]
    meta = plan["meta"]
    nc = bass.Bass()
    idx = nc.declare_dram_parameter("idx", [128, 8 * T], i16, isOutput=False)
    ind = nc.declare_dram_parameter("ind", [128, T * GW], bf16, isOutput=False)
    xg = nc.declare_dram_parameter("xg", [128, T * 128], bf16, isOutput=False)
    xT = nc.declare_dram_parameter("xT", [128, PADN], bf16, isOutput=False)
    wl = nc.declare_dram_parameter("wl", [3 * 128, 128], bf16, isOutput=False)
    wr = nc.declare_dram_parameter("wr", [3 * 128, 128], bf16, isOutput=False)
    bb = nc.declare_dram_parameter("bb", [128, 3], f32, isOutput=False)
    out = nc.declare_dram_parameter("out", [128, 1], f32, isOutput=True)
    htdbg = [nc.declare_dram_parameter(f"ht{li}", [128, PADN], bf16,
                                       isOutput=True)
             for li in range(3)] if debug else None

    shard = [[nc.dram_tensor(f"shard{li}_{qq}", [QSIZES[qq], D], bf16)
              for qq in range(NQ)] for li in range(2)]
    tables = [[nc.dram_tensor(f"table{li}_{qq}", [P * QSIZES[qq], D], bf16,
                              addr_space="Shared")
               for qq in range(NQ)] for li in range(2)]

    Relu = mybir.ActivationFunctionType.Relu
    GMAX = max(m["ntiles"] for m in meta)

    with tile.TileContext(nc) as tc, ExitStack() as ctx:
        nc.gpsimd.load_library(library_config.mlp)
        nreg = {}  # one register per distinct num_idxs value
        for m in meta:
            for (_, _, n) in m["pieces"]:
                if n * 128 not in nreg:
                    nreg[n * 128] = nc.gpsimd.to_reg(n * 128)
        res = ctx.enter_context(tc.tile_pool(name="res", bufs=1))
        gp = ctx.enter_context(tc.tile_pool(name="g", bufs=3))
        aggp = ctx.enter_context(tc.tile_pool(name="agg", bufs=2))
        rowp = ctx.enter_context(tc.tile_pool(name="row", bufs=2))
        psg = ctx.enter_context(tc.tile_pool(name="psg", bufs=2, space="PSUM"))
        psh = ctx.enter_context(tc.tile_pool(name="psh", bufs=2, space="PSUM"))
        pst = ctx.enter_context(tc.tile_pool(name="pst", bufs=2, space="PSUM"))

        # ---- residents
        ind_sb = res.tile([128, T * GW], bf16)
        nc.sync.dma_start(ind_sb[:], ind[:])
        idx_sb = res.tile([128, 8 * T], i16)
        nc.sync.dma_start(idx_sb[:], idx[:])
        HT = res.tile([128, PADN], bf16)
        nc.sync.dma_start(HT[:], xT[:])
        wl_sb = res.tile([128, 3 * 128], bf16)
        nc.sync.dma_start(wl_sb[:].rearrange("p (l f) -> p l f", l=3),
                          wl[:].rearrange("(l p) f -> p l f", p=128))
        wr_sb = res.tile([128, 3 * 128], bf16)
        nc.sync.dma_start(wr_sb[:].rearrange("p (l f) -> p l f", l=3),
                          wr[:].rearrange("(l p) f -> p l f", p=128))
        bb_sb = res.tile([128, 3], f32)
        nc.sync.dma_start(bb_sb[:], bb[:])
        pool_st = res.tile([128, 3 * NCHUNK + 1], f32)
        ident = res.tile([128, 128], bf16)
        from concourse.masks import make_identity
        make_identity(nc, ident[:])

        for li in range(3):
            for ci in range(NCHUNK):
                glo, ghi, cs, w = chunks[ci]
                m = meta[ci]
                tb, nt = m["tbase"], m["ntiles"]
                G = gp.tile([128, GMAX * 128], bf16, tag="g")
                if li == 0:
                    nc.sync.dma_start(G[:, :nt * 128],
                                      xg[:, tb * 128:(tb + nt) * 128])
                else:
                    for (qq, toff, n) in m["pieces"]:
                        nc.gpsimd.dma_gather(
                            out_ap=G[:, toff * 128:(toff + n) * 128]
                            .rearrange("p (j f) -> p j f", f=128),
                            in_ap=tables[li - 1][qq][:],
                            idxs_ap=idx_sb[:, 8 * (tb + toff):8 * (tb + toff + n)],
                            num_idxs=n * 128, num_idxs_reg=nreg[n * 128],
                            elem_size=128)
                ps = psg.tile([128, 512], f32, tag="psg")
                # PSUM accumulation chains must be consecutive per region:
                # issue group-major (stable keeps start->stop order)
                for (toff, cb, st, sp) in sorted(m["mm"], key=lambda z: z[1]):
                    nc.tensor.matmul(
                        ps[:, cb:cb + GW],
                        lhsT=G[:, toff * 128:(toff + 1) * 128],
                        rhs=ind_sb[:, (tb + toff) * GW:(tb + toff + 1) * GW],
                        start=st, stop=sp)
                agg = aggp.tile([128, 512], bf16, tag="agg")
                nc.vector.tensor_copy(agg[:, :w], ps[:, :w])

                ph = psh.tile([128, 512], f32, tag="psh")
                nc.tensor.matmul(ph[:, :w], lhsT=wl_sb[:, li * 128:(li + 1) * 128],
                                 rhs=agg[:, :w], start=True, stop=False)
                nc.tensor.matmul(ph[:, :w], lhsT=wr_sb[:, li * 128:(li + 1) * 128],
                                 rhs=HT[:, cs:cs + w], start=False, stop=True)
                pcol = pool_st[:, li * NCHUNK + ci:li * NCHUNK + ci + 1]
                if ci < NCHUNK - 1:
                    nc.scalar.activation(HT[:, cs:cs + w], ph[:, :w], Relu,
                                         bias=bb_sb[:, li:li + 1], accum_out=pcol)
                else:
                    nv = NPC - cs  # valid columns in last chunk
                    nc.scalar.activation(HT[:, cs:cs + nv], ph[:, :nv], Relu,
                                         bias=bb_sb[:, li:li + 1], accum_out=pcol)
                    nc.scalar.activation(HT[:, cs + nv:cs + w],
                                         ph[:, nv:w], Relu,
                                         bias=bb_sb[:, li:li + 1])
                if li < 2:
                    row = rowp.tile([128, 512], bf16, tag="row")
                    for j in range(w // 128):
                        pt = pst.tile([128, 128], bf16, tag="pst")
                        nc.tensor.transpose(
                            pt[:], HT[:, cs + j * 128:cs + (j + 1) * 128],
                            ident[:])
                        nc.vector.tensor_copy(row[:, j * 128:(j + 1) * 128],
                                              pt[:])
                    for (qq, qr, cnt, coff) in _quarter_pieces(cs, w):
                        nc.sync.dma_start(
                            shard[li][qq][qr:qr + cnt, :]
                            .rearrange("(j p) f -> p j f", p=128),
                            row[:, coff:coff + cnt]
                            .rearrange("p (j f) -> p j f", f=128))
                    # early-issue the AllGather for each quarter as soon as
                    # its last shard rows are written (overlaps with the
                    # remaining chunks' compute)
                    for qq in range(NQ):
                        if (QSTART[qq] + QSIZES[qq] - 1) // 512 == ci:
                            if sim_local_collectives:
                                for cc in range(P):
                                    nc.sync.dma_start(
                                        tables[li][qq][cc * QSIZES[qq]:
                                                       (cc + 1) * QSIZES[qq], :],
                                        shard[li][qq][:])
                            else:
                                nc.gpsimd.collective_compute(
                                    "AllGather", mybir.AluOpType.bypass,
                                    ins=[shard[li][qq][:]],
                                    outs=[tables[li][qq][:]],
                                    replica_groups=[list(range(P))])
            if debug:
                nc.sync.dma_start(htdbg[li][:], HT[:])

        nc.vector.reduce_sum(pool_st[:, 3 * NCHUNK:], pool_st[:, :3 * NCHUNK],
                             axis=mybir.AxisListType.X)
        outp = res.tile([128, 1], f32)
        nc.vector.tensor_copy(outp[:], pool_st[:, 3 * NCHUNK:])
        nc.sync.dma_start(out[:], outp[:])
    _split_multi_waits(nc)
    _fill_reload_bytes(nc)
    return nc


# ---------------------------------------------------------------------------
def _make_runner(nc, n_cores=P):
    import jax
    from jax.sharding import Mesh, PartitionSpec
    try:
        from jax.experimental.shard_map import shard_map
    except ImportError:
        from jax.shard_map import shard_map
    from concourse import bass2jax
    from concourse.bass2jax import _bass_exec_p, partition_id_tensor

    bass2jax.install_neuronx_cc_hook()
    partition_name = nc.partition_id_tensor.name if nc.partition_id_tensor else None
    in_names, out_names, out_avals, zero_outs = [], [], [], []
    for alloc in nc.m.functions[0].allocations:
        if not isinstance(alloc, mybir.MemoryLocationSet):
            continue
        name = alloc.memorylocations[0].name
        if alloc.kind == "ExternalInput":
            if name != partition_name:
                in_names.append(name)
        elif alloc.kind == "ExternalOutput":
            out_names.append(name)
            shape = tuple(alloc.tensor_shape)
            dtype = mybir.dt.np(alloc.dtype)
            out_avals.append(jax.core.ShapedArray(shape, dtype))
            zero_outs.append(np.zeros(shape, dtype))
    n_params = len(in_names)
    in_names_all = list(in_names) + list(out_names)
    if partition_name is not None:
        in_names_all.append(partition_name)

    def _body(*args):
        operands = list(args)
        if partition_name is not None:
            operands.append(partition_id_tensor())
        return tuple(_bass_exec_p.bind(
            *operands, out_avals=tuple(out_avals), in_names=tuple(in_names_all),
            out_names=tuple(out_names), lowering_input_output_aliases=(),
            sim_require_finite=True, sim_require_nnan=True, nc=nc))

    devices = jax.devices()[:n_cores]
    mesh = Mesh(np.asarray(devices), ("core",))
    nspec = n_params + len(out_names)
    sharded = jax.jit(
        shard_map(_body, mesh=mesh,
                  in_specs=(PartitionSpec("core"),) * nspec,
                  out_specs=(PartitionSpec("core"),) * len(out_names),
                  check_rep=False),
        keep_unused=True)

    def run(in_maps):
        per_core = [[np.asarray(m[name]) for name in in_names] for m in in_maps]
        concat_in = [np.concatenate([per_core[c][i] for c in range(n_cores)], axis=0)
                     for i in range(n_params)]
        concat_zeros = [np.zeros((n_cores * z.shape[0], *z.shape[1:]), z.dtype)
                        for z in zero_outs]
        args = concat_in + concat_zeros
        out_arrs = sharded(*args)
        jax.block_until_ready(out_arrs)
        return [{name: np.asarray(out_arrs[i]).reshape(n_cores,
                                                       *out_avals[i].shape)[c]
                 for i, name in enumerate(out_names)}
                for c in range(n_cores)], (sharded, args)
    return run


_CACHE = {}


def kernel(x, lin_l_w, lin_l_b, lin_r_w, bn_w, bn_b,
           fc1_w, fc1_b, fc2_w, fc2_b, fc3_w, fc3_b, edge_index):
    x = np.asarray(x, np.float32)
    per_core, plan, _ = _host_prep(x, edge_index)
    wlw, wrw, bbw = _fold_weights(lin_l_w, lin_l_b, lin_r_w, bn_w, bn_b)
    wl_in = np.ascontiguousarray(wlw.reshape(3 * 128, 128))
    wr_in = np.ascontiguousarray(wrw.reshape(3 * 128, 128))

    key = (plan["T"],) + tuple(
        (m["tbase"],) + tuple(m["pieces"]) for m in plan["meta"])
    if key not in _CACHE:
        nc = _build(plan)
        _CACHE[key] = _make_runner(nc)
    run = _CACHE[key]

    in_maps = [{**per_core[c], "wl": wl_in, "wr": wr_in, "bb": bbw}
               for c in range(P)]
    res, _ = run(in_maps)

    g = x.sum(axis=0, dtype=np.float64).astype(np.float32)
    for c in range(P):
        g = g + res[c]["out"][:, 0]

    fc1_w = np.asarray(fc1_w, np.float32); fc1_b = np.asarray(fc1_b, np.float32)
    fc2_w = np.asarray(fc2_w, np.float32); fc2_b = np.asarray(fc2_b, np.float32)
    fc3_w = np.asarray(fc3_w, np.float32); fc3_b = np.asarray(fc3_b, np.float32)
    h = np.maximum(g @ fc1_w.T + fc1_b, 0.0)
    h = np.maximum(h @ fc2_w.T + fc2_b, 0.0)
    o = h @ fc3_w.T + fc3_b
    return o[None, :].astype(np.float32)
